# revision 12
# baseline (speedup 1.0000x reference)
"""3x3 median filter (reflect padding) on Trainium2, 8-core data parallel.

Layout (per core, 4 images):
  partition p = b*32 + g
    b in 0..3  : image index within the core's batch shard
    g in 0..31 : group of 7 consecutive output rows
  linear(p) = p*7*W*C addresses (b,g) jointly (the strides nest
  perfectly), so one 3-dim access pattern spans all 128 partitions.

All device compute is fp16 (host converts): 2-byte packed operands put
the DVE in its 2x perf mode and halve DMA traffic; max quantization
error ~2.5e-4 against a 2e-2 tolerance.

A single 9-row slab per partition (rows 7g-1 .. 7g+7) is loaded once
(9/7 traffic amplification instead of 11/7 for two chunks), with halo
rows fetched by one 127-partition DMA plus 4 small reflect overwrites
at image boundaries.  Stage 1 is split into two ranges so compute
starts after the first 4 slab rows land; stage 2 runs as whole-slab
ops (max instruction size amortizes the DVE's fixed per-instruction
cost); the final merge + store stream out in row pairs so the last
store is the only exposed tail.

Median of 9 = med3( max3(col_lows), med3(col_meds), min3(col_highs) )
with each vertical column triple sorted once and shared across the three
horizontally adjacent windows.  Horizontal neighbor access is a +-3
float shift inside each row; the image's first/last output columns are
recomputed exactly with narrow per-column ops and overwritten before
the store.
"""

import sys

if "/opt/trn_rl_repo" not in sys.path:
    sys.path.insert(0, "/opt/trn_rl_repo")

import numpy as np

import concourse.bass as bass  # noqa: F401
import concourse.tile as tile
from concourse import bacc, mybir
from concourse.ap import AP
from concourse.bass_utils import run_bass_kernel_spmd

F32 = mybir.dt.float32
F16 = mybir.dt.float16
MIN = mybir.AluOpType.min
MAX = mybir.AluOpType.max

B, H, W, C = 32, 224, 224, 3
NCORES = 8
BPC = B // NCORES      # 4 images per core
NG, GR = 32, 7         # row-groups per image, rows per group
WC = W * C             # 672 floats per image row
IMG = H * WC
PS = GR * WC           # 4704: per-partition linear stride
R = GR                 # 7 output rows per partition
N = R * WC             # 4704 output floats per partition
SRR = R + 2            # 9 slab rows

_CACHE = {}


def _build_kernel(tc, y, x):
    nc = tc.nc
    qa, qb = nc.sync, nc.scalar

    with tc.tile_pool(name="sb", bufs=1) as sb:
        S = sb.tile([128, SRR, WC], F16, tag="s", name="S")

        # ---- loads: 3 sub-waves on 3 HW-DGE queues -----------------
        # Each sub-wave is a contiguous multi-row packet per partition.
        # A1 starts one DRAM row early to absorb the top halo: the
        # image-boundary partitions (p = b*32) read a wrong top row and
        # are patched with one 4-partition DMA of their reflect row
        # (own row 1).  Same at the bottom for wave B (p = b*32+31,
        # reflect row 222).  p=0 / p=127 cannot over-read at the tensor
        # ends and load separately.  The vector queue only carries
        # wave-A pieces (issued before its compute stream starts).
        def rows(q, p0, p1, dram_row, s0, nr):
            q.dma_start(S[p0:p1, s0:s0 + nr, :],
                        AP(x.tensor, p0 * PS + dram_row * WC,
                           [[PS, p1 - p0], [1, nr * WC]]))

        # wave A1: slab rows 0..1 (local -1..0)
        rows(qa, 1, 64, -1, 0, 2)
        rows(qb, 64, 128, -1, 0, 2)
        qb.dma_start(S[0:1, 1:2, :],         # p0 local row 0
                     AP(x.tensor, 0, [[1, WC]]))
        # reflect patch for the top halo at p = 0,32,64,96
        qa.dma_start(S[0:128:32, 0:1, :],
                     AP(x.tensor, WC, [[IMG, 4], [1, WC]]))
        # wave A2: slab rows 2..4 (local 1..3); gpsimd (SWDGE) carries
        # a third piece of each later wave -- slower per packet, but it
        # adds parallel bandwidth off the critical queues
        qg = nc.gpsimd
        rows(qa, 0, 43, 1, 2, 3)
        rows(qb, 64, 107, 1, 2, 3)
        rows(qg, 43, 64, 1, 2, 3)
        rows(qg, 107, 128, 1, 2, 3)
        # wave B: slab rows 5..8 (local 4..7)
        rows(qa, 0, 43, 4, 5, 4)
        rows(qb, 64, 107, 4, 5, 4)
        rows(qg, 43, 64, 4, 5, 4)
        rows(qg, 107, 127, 4, 5, 4)
        qb.dma_start(S[127:128, 5:8, :],     # p127 local rows 4..6
                     AP(x.tensor, 127 * PS + 4 * WC, [[1, 3 * WC]]))
        # reflect patch for the bottom halo at p = 31,63,95,127
        qa.dma_start(S[31:128:32, 8:9, :],
                     AP(x.tensor, (H - 2) * WC, [[IMG, 4], [1, WC]]))

        Sf = S.rearrange("p r f -> p (r f)")

        # ---- stage 1: vertical column sort (flat over slab) --------
        # P/Q = min/max of vertically adjacent rows; LO/MED/HI finish
        # the column triple with the third row.  Three range splits so
        # compute starts as soon as sub-wave A1 lands.
        P = sb.tile([128, N], F16, tag="p", name="P")
        Q = sb.tile([128, N], F16, tag="q", name="Q")
        LO = sb.tile([128, R, WC], F16, tag="lo", name="LO")
        T = sb.tile([128, R, WC], F16, tag="t", name="T")
        LOf = LO.rearrange("p r f -> p (r f)")
        Tf = T.rearrange("p r f -> p (r f)")

        def s1_pq(fa, fb):
            nc.vector.tensor_tensor(P[:, fa:fb], Sf[:, fa:fb],
                                    Sf[:, fa + WC:fb + WC], MIN)
            nc.vector.tensor_tensor(Q[:, fa:fb], Sf[:, fa:fb],
                                    Sf[:, fa + WC:fb + WC], MAX)

        def s1_cols(fa, fb):
            nc.vector.tensor_tensor(LOf[:, fa:fb], P[:, fa:fb],
                                    Sf[:, fa + 2 * WC:fb + 2 * WC], MIN)
            nc.vector.tensor_tensor(Tf[:, fa:fb], Q[:, fa:fb],
                                    Sf[:, fa + 2 * WC:fb + 2 * WC], MIN)
            # MED (in T): max(P, min(Q, S+2))
            nc.vector.tensor_tensor(Tf[:, fa:fb], P[:, fa:fb],
                                    Tf[:, fa:fb], MAX)
            # HI (in Q): max(Q, S+2)
            nc.vector.tensor_tensor(Q[:, fa:fb], Q[:, fa:fb],
                                    Sf[:, fa + 2 * WC:fb + 2 * WC], MAX)

        s1_pq(0, WC)              # needs slab rows 0..1  (A1)
        s1_pq(WC, 4 * WC)         # needs slab rows 1..4  (A2)
        s1_cols(0, 2 * WC)        # needs slab rows 0..3  (A2)
        s1_pq(4 * WC, N)          # needs slab rows 4..7  (B)
        s1_cols(2 * WC, N)        # needs slab rows 2..8  (B)
        HI = Q.rearrange("p (r f) -> p r f", f=WC)

        M1 = sb.tile([128, R, WC], F16, tag="m1", name="M1")

        # ---- exact first/last output columns (reflect), both at once
        # col 0: window cols (1,0,1) -> med3(max(lo0,lo1), med1,
        # min(hi0,hi1)); col 223: window cols (222,223,222).
        L4 = LO.rearrange("p r (a c) -> p r a c", a=W, c=C)
        H4 = HI.rearrange("p r (a c) -> p r a c", a=W, c=C)
        T4 = T.rearrange("p r (a c) -> p r a c", a=W, c=C)
        M4 = M1.rearrange("p r (a c) -> p r a c", a=W, c=C)
        lo_o = L4[:, :, 0:W:W - 1, :]      # cols {0, 223}
        lo_i = L4[:, :, 1:W:W - 3, :]      # cols {1, 222}
        hi_o = H4[:, :, 0:W:W - 1, :]
        hi_i = H4[:, :, 1:W:W - 3, :]
        be = T4[:, :, 1:W:W - 3, :]        # med of inner col
        ae = sb.tile([128, R, 2, C], F16, tag="ae", name="ae")
        ce = sb.tile([128, R, 2, C], F16, tag="ce", name="ce")
        me = sb.tile([128, R, 2, C], F16, tag="me", name="me")
        nc.vector.tensor_tensor(ae[:], lo_o, lo_i, MAX)
        nc.vector.tensor_tensor(ce[:], hi_o, hi_i, MIN)
        nc.vector.tensor_tensor(me[:], ae[:], be, MIN)
        nc.vector.tensor_tensor(ae[:], ae[:], be, MAX)
        nc.vector.tensor_tensor(ce[:], ae[:], ce[:], MIN)
        nc.vector.tensor_tensor(M4[:, :, 0:W:W - 1, :], me[:], ce[:], MAX)

        # ---- stage 2: horizontal merge in 2 row groups, final med3
        # chain + store in 3 groups so stores stream while the tail
        # group is still computing
        E = WC - 3   # 669
        D = WC - 6   # 666
        U = sb.tile([128, R, WC], F16, tag="u", name="U")
        V = sb.tile([128, R, WC], F16, tag="v", name="V")
        Sm = sb.tile([128, R, WC], F16, tag="sm", name="Sm")
        Tm = sb.tile([128, R, WC], F16, tag="tm", name="Tm")
        MT = sb.tile([128, R, WC], F16, tag="mt", name="MT")
        A = U   # max3 of lows
        Cc = V  # min3 of highs
        Bm = Sm  # med3 of meds

        for (ra, rb) in ((0, 4), (4, 7)):
            nc.vector.tensor_tensor(U[:, ra:rb, 0:E], LO[:, ra:rb, 0:E],
                                    LO[:, ra:rb, 3:WC], MAX)
            nc.vector.tensor_tensor(U[:, ra:rb, 0:D], U[:, ra:rb, 0:D],
                                    LO[:, ra:rb, 6:WC], MAX)
            nc.vector.tensor_tensor(V[:, ra:rb, 0:E], HI[:, ra:rb, 0:E],
                                    HI[:, ra:rb, 3:WC], MIN)
            nc.vector.tensor_tensor(V[:, ra:rb, 0:D], V[:, ra:rb, 0:D],
                                    HI[:, ra:rb, 6:WC], MIN)
            nc.vector.tensor_tensor(Sm[:, ra:rb, 0:E], T[:, ra:rb, 0:E],
                                    T[:, ra:rb, 3:WC], MIN)
            nc.vector.tensor_tensor(Tm[:, ra:rb, 0:E], T[:, ra:rb, 0:E],
                                    T[:, ra:rb, 3:WC], MAX)
            nc.vector.tensor_tensor(Tm[:, ra:rb, 0:D], Tm[:, ra:rb, 0:D],
                                    T[:, ra:rb, 6:WC], MIN)
            nc.vector.tensor_tensor(Sm[:, ra:rb, 0:D], Sm[:, ra:rb, 0:D],
                                    Tm[:, ra:rb, 0:D], MAX)

        for (ra, rb) in ((0, 4), (4, 6), (6, 7)):
            nc.vector.tensor_tensor(MT[:, ra:rb, 0:D], A[:, ra:rb, 0:D],
                                    Bm[:, ra:rb, 0:D], MIN)
            nc.vector.tensor_tensor(A[:, ra:rb, 0:D], A[:, ra:rb, 0:D],
                                    Bm[:, ra:rb, 0:D], MAX)
            nc.vector.tensor_tensor(Cc[:, ra:rb, 0:D], A[:, ra:rb, 0:D],
                                    Cc[:, ra:rb, 0:D], MIN)
            nc.vector.tensor_tensor(M1[:, ra:rb, 3:WC - 3],
                                    MT[:, ra:rb, 0:D],
                                    Cc[:, ra:rb, 0:D], MAX)
            for (p0, p1, q) in ((0, 64, qa), (64, 128, qb)):
                dst = AP(y.tensor, p0 * PS + ra * WC,
                         [[PS, p1 - p0], [WC, rb - ra], [1, WC]])
                q.dma_start(dst, M1[p0:p1, ra:rb, :])


def _build():
    if "nc" in _CACHE:
        return _CACHE["nc"]
    nc = bacc.Bacc("TRN2", target_bir_lowering=False, debug=False)
    x = nc.dram_tensor("x", [BPC, H, W, C], F16, kind="ExternalInput").ap()
    y = nc.dram_tensor("y", [BPC, H, W, C], F16, kind="ExternalOutput").ap()
    with tile.TileContext(nc) as tc:
        _build_kernel(tc, y, x)
    nc.compile()
    _CACHE["nc"] = nc
    return nc


def run(input_batch, **spmd_kwargs):
    nc = _build()
    xh = np.ascontiguousarray(input_batch).astype(np.float16)
    in_maps = [
        {"x": np.ascontiguousarray(xh[i * BPC:(i + 1) * BPC])}
        for i in range(NCORES)
    ]
    res = run_bass_kernel_spmd(nc, in_maps, list(range(NCORES)), **spmd_kwargs)
    out = np.concatenate([r["y"] for r in res.results],
                         axis=0).astype(np.float32)
    return out, res


def kernel(input_batch):
    out, _ = run(np.asarray(input_batch))
    return out


# revision 13
# speedup vs baseline: 1.3584x; 1.3584x over previous
"""3x3 median filter (reflect padding) on Trainium2, 8-core data parallel.

Layout (per core, 4 images):
  partition p = b*32 + g
    b in 0..3  : image index within the core's batch shard
    g in 0..31 : group of 7 consecutive output rows
  linear(p) = p*7*W*C addresses (b,g) jointly (the strides nest
  perfectly), so one 3-dim access pattern spans all 128 partitions.

All device compute is fp16 (host converts): 2-byte packed operands put
the DVE in its 2x perf mode and halve DMA traffic; max quantization
error ~2.5e-4 against a 2e-2 tolerance.

A single 9-row slab per partition (rows 7g-1 .. 7g+7) is loaded once
(9/7 traffic amplification instead of 11/7 for two chunks), with halo
rows fetched by one 127-partition DMA plus 4 small reflect overwrites
at image boundaries.  Stage 1 is split into two ranges so compute
starts after the first 4 slab rows land; stage 2 runs as whole-slab
ops (max instruction size amortizes the DVE's fixed per-instruction
cost); the final merge + store stream out in row pairs so the last
store is the only exposed tail.

Median of 9 = med3( max3(col_lows), med3(col_meds), min3(col_highs) )
with each vertical column triple sorted once and shared across the three
horizontally adjacent windows.  Horizontal neighbor access is a +-3
float shift inside each row; the image's first/last output columns are
recomputed exactly with narrow per-column ops and overwritten before
the store.
"""

import sys

if "/opt/trn_rl_repo" not in sys.path:
    sys.path.insert(0, "/opt/trn_rl_repo")

import numpy as np

import concourse.bass as bass  # noqa: F401
import concourse.tile as tile
from concourse import bacc, mybir
from concourse.ap import AP
from concourse.bass_utils import run_bass_kernel_spmd

F32 = mybir.dt.float32
F16 = mybir.dt.float16
MIN = mybir.AluOpType.min
MAX = mybir.AluOpType.max

B, H, W, C = 32, 224, 224, 3
NCORES = 8
BPC = B // NCORES      # 4 images per core
NG, GR = 32, 7         # row-groups per image, rows per group
WC = W * C             # 672 floats per image row
IMG = H * WC
PS = GR * WC           # 4704: per-partition linear stride
R = GR                 # 7 output rows per partition
N = R * WC             # 4704 output floats per partition
SRR = R + 2            # 9 slab rows

_CACHE = {}


def _build_kernel(tc, y, x):
    nc = tc.nc
    qa, qb = nc.sync, nc.scalar

    with tc.tile_pool(name="sb", bufs=1) as sb:
        S = sb.tile([128, SRR, WC], F16, tag="s", name="S")

        # ---- loads: 3 sub-waves on 3 HW-DGE queues -----------------
        # Each sub-wave is a contiguous multi-row packet per partition.
        # A1 starts one DRAM row early to absorb the top halo: the
        # image-boundary partitions (p = b*32) read a wrong top row and
        # are patched with one 4-partition DMA of their reflect row
        # (own row 1).  Same at the bottom for wave B (p = b*32+31,
        # reflect row 222).  p=0 / p=127 cannot over-read at the tensor
        # ends and load separately.  The vector queue only carries
        # wave-A pieces (issued before its compute stream starts).
        def rows(q, p0, p1, dram_row, s0, nr):
            q.dma_start(S[p0:p1, s0:s0 + nr, :],
                        AP(x.tensor, p0 * PS + dram_row * WC,
                           [[PS, p1 - p0], [1, nr * WC]]))

        # wave A1: slab rows 0..1 (local -1..0)
        rows(qa, 1, 64, -1, 0, 2)
        rows(qb, 64, 128, -1, 0, 2)
        qb.dma_start(S[0:1, 1:2, :],         # p0 local row 0
                     AP(x.tensor, 0, [[1, WC]]))
        # reflect patch for the top halo at p = 0,32,64,96
        qa.dma_start(S[0:128:32, 0:1, :],
                     AP(x.tensor, WC, [[IMG, 4], [1, WC]]))
        # wave A2: slab rows 2..4 (local 1..3)
        rows(qa, 0, 64, 1, 2, 3)
        rows(qb, 64, 128, 1, 2, 3)
        # wave B: slab rows 5..8 (local 4..7)
        rows(qa, 0, 64, 4, 5, 4)
        rows(qb, 64, 127, 4, 5, 4)
        qb.dma_start(S[127:128, 5:8, :],     # p127 local rows 4..6
                     AP(x.tensor, 127 * PS + 4 * WC, [[1, 3 * WC]]))
        # reflect patch for the bottom halo at p = 31,63,95,127
        qa.dma_start(S[31:128:32, 8:9, :],
                     AP(x.tensor, (H - 2) * WC, [[IMG, 4], [1, WC]]))

        Sf = S.rearrange("p r f -> p (r f)")

        # ---- stage 1: vertical column sort (flat over slab) --------
        # P/Q = min/max of vertically adjacent rows; LO/MED/HI finish
        # the column triple with the third row.  Three range splits so
        # compute starts as soon as sub-wave A1 lands.
        P = sb.tile([128, N], F16, tag="p", name="P")
        Q = sb.tile([128, N], F16, tag="q", name="Q")
        LO = sb.tile([128, R, WC], F16, tag="lo", name="LO")
        T = sb.tile([128, R, WC], F16, tag="t", name="T")
        LOf = LO.rearrange("p r f -> p (r f)")
        Tf = T.rearrange("p r f -> p (r f)")

        def s1_pq(fa, fb):
            nc.vector.tensor_tensor(P[:, fa:fb], Sf[:, fa:fb],
                                    Sf[:, fa + WC:fb + WC], MIN)
            nc.vector.tensor_tensor(Q[:, fa:fb], Sf[:, fa:fb],
                                    Sf[:, fa + WC:fb + WC], MAX)

        def s1_cols(fa, fb):
            nc.vector.tensor_tensor(LOf[:, fa:fb], P[:, fa:fb],
                                    Sf[:, fa + 2 * WC:fb + 2 * WC], MIN)
            nc.vector.tensor_tensor(Tf[:, fa:fb], Q[:, fa:fb],
                                    Sf[:, fa + 2 * WC:fb + 2 * WC], MIN)
            # MED (in T): max(P, min(Q, S+2))
            nc.vector.tensor_tensor(Tf[:, fa:fb], P[:, fa:fb],
                                    Tf[:, fa:fb], MAX)
            # HI (in Q): max(Q, S+2)
            nc.vector.tensor_tensor(Q[:, fa:fb], Q[:, fa:fb],
                                    Sf[:, fa + 2 * WC:fb + 2 * WC], MAX)

        s1_pq(0, WC)              # needs slab rows 0..1  (A1)
        s1_pq(WC, 4 * WC)         # needs slab rows 1..4  (A2)
        s1_cols(0, 2 * WC)        # needs slab rows 0..3  (A2)
        s1_pq(4 * WC, N)          # needs slab rows 4..7  (B)
        s1_cols(2 * WC, N)        # needs slab rows 2..8  (B)
        HI = Q.rearrange("p (r f) -> p r f", f=WC)

        M1 = sb.tile([128, R, WC], F16, tag="m1", name="M1")

        # ---- exact first/last output columns (reflect), both at once
        # col 0: window cols (1,0,1) -> med3(max(lo0,lo1), med1,
        # min(hi0,hi1)); col 223: window cols (222,223,222).
        L4 = LO.rearrange("p r (a c) -> p r a c", a=W, c=C)
        H4 = HI.rearrange("p r (a c) -> p r a c", a=W, c=C)
        T4 = T.rearrange("p r (a c) -> p r a c", a=W, c=C)
        M4 = M1.rearrange("p r (a c) -> p r a c", a=W, c=C)
        lo_o = L4[:, :, 0:W:W - 1, :]      # cols {0, 223}
        lo_i = L4[:, :, 1:W:W - 3, :]      # cols {1, 222}
        hi_o = H4[:, :, 0:W:W - 1, :]
        hi_i = H4[:, :, 1:W:W - 3, :]
        be = T4[:, :, 1:W:W - 3, :]        # med of inner col
        ae = sb.tile([128, R, 2, C], F16, tag="ae", name="ae")
        ce = sb.tile([128, R, 2, C], F16, tag="ce", name="ce")
        me = sb.tile([128, R, 2, C], F16, tag="me", name="me")
        nc.vector.tensor_tensor(ae[:], lo_o, lo_i, MAX)
        nc.vector.tensor_tensor(ce[:], hi_o, hi_i, MIN)
        nc.vector.tensor_tensor(me[:], ae[:], be, MIN)
        nc.vector.tensor_tensor(ae[:], ae[:], be, MAX)
        nc.vector.tensor_tensor(ce[:], ae[:], ce[:], MIN)
        nc.vector.tensor_tensor(M4[:, :, 0:W:W - 1, :], me[:], ce[:], MAX)

        # ---- stage 2: horizontal merge in 2 row groups, final med3
        # chain + store in 3 groups so stores stream while the tail
        # group is still computing
        E = WC - 3   # 669
        D = WC - 6   # 666
        U = sb.tile([128, R, WC], F16, tag="u", name="U")
        V = sb.tile([128, R, WC], F16, tag="v", name="V")
        Sm = sb.tile([128, R, WC], F16, tag="sm", name="Sm")
        Tm = sb.tile([128, R, WC], F16, tag="tm", name="Tm")
        MT = sb.tile([128, R, WC], F16, tag="mt", name="MT")
        A = U   # max3 of lows
        Cc = V  # min3 of highs
        Bm = Sm  # med3 of meds

        for (ra, rb) in ((0, 4), (4, 7)):
            nc.vector.tensor_tensor(U[:, ra:rb, 0:E], LO[:, ra:rb, 0:E],
                                    LO[:, ra:rb, 3:WC], MAX)
            nc.vector.tensor_tensor(U[:, ra:rb, 0:D], U[:, ra:rb, 0:D],
                                    LO[:, ra:rb, 6:WC], MAX)
            nc.vector.tensor_tensor(V[:, ra:rb, 0:E], HI[:, ra:rb, 0:E],
                                    HI[:, ra:rb, 3:WC], MIN)
            nc.vector.tensor_tensor(V[:, ra:rb, 0:D], V[:, ra:rb, 0:D],
                                    HI[:, ra:rb, 6:WC], MIN)
            nc.vector.tensor_tensor(Sm[:, ra:rb, 0:E], T[:, ra:rb, 0:E],
                                    T[:, ra:rb, 3:WC], MIN)
            nc.vector.tensor_tensor(Tm[:, ra:rb, 0:E], T[:, ra:rb, 0:E],
                                    T[:, ra:rb, 3:WC], MAX)
            nc.vector.tensor_tensor(Tm[:, ra:rb, 0:D], Tm[:, ra:rb, 0:D],
                                    T[:, ra:rb, 6:WC], MIN)
            nc.vector.tensor_tensor(Sm[:, ra:rb, 0:D], Sm[:, ra:rb, 0:D],
                                    Tm[:, ra:rb, 0:D], MAX)

        for (ra, rb) in ((0, 4), (4, 6), (6, 7)):
            nc.vector.tensor_tensor(MT[:, ra:rb, 0:D], A[:, ra:rb, 0:D],
                                    Bm[:, ra:rb, 0:D], MIN)
            nc.vector.tensor_tensor(A[:, ra:rb, 0:D], A[:, ra:rb, 0:D],
                                    Bm[:, ra:rb, 0:D], MAX)
            nc.vector.tensor_tensor(Cc[:, ra:rb, 0:D], A[:, ra:rb, 0:D],
                                    Cc[:, ra:rb, 0:D], MIN)
            nc.vector.tensor_tensor(M1[:, ra:rb, 3:WC - 3],
                                    MT[:, ra:rb, 0:D],
                                    Cc[:, ra:rb, 0:D], MAX)
            for (p0, p1, q) in ((0, 64, qa), (64, 128, qb)):
                dst = AP(y.tensor, p0 * PS + ra * WC,
                         [[PS, p1 - p0], [WC, rb - ra], [1, WC]])
                q.dma_start(dst, M1[p0:p1, ra:rb, :])


def _build():
    if "nc" in _CACHE:
        return _CACHE["nc"]
    nc = bacc.Bacc("TRN2", target_bir_lowering=False, debug=False)
    x = nc.dram_tensor("x", [BPC, H, W, C], F16, kind="ExternalInput").ap()
    y = nc.dram_tensor("y", [BPC, H, W, C], F16, kind="ExternalOutput").ap()
    with tile.TileContext(nc) as tc:
        _build_kernel(tc, y, x)
    nc.compile()
    _CACHE["nc"] = nc
    return nc


def run(input_batch, **spmd_kwargs):
    nc = _build()
    xh = np.ascontiguousarray(input_batch).astype(np.float16)
    in_maps = [
        {"x": np.ascontiguousarray(xh[i * BPC:(i + 1) * BPC])}
        for i in range(NCORES)
    ]
    res = run_bass_kernel_spmd(nc, in_maps, list(range(NCORES)), **spmd_kwargs)
    out = np.concatenate([r["y"] for r in res.results],
                         axis=0).astype(np.float32)
    return out, res


def kernel(input_batch):
    out, _ = run(np.asarray(input_batch))
    return out
